# revision 3
# baseline (speedup 1.0000x reference)
"""Trainium2 Bass kernel for a 2-layer GCN (BayesianGCN in eval mode).

Math: with dinv = rsqrt(in_degree + 2) the symmetric GCN normalization
factors per node (norm_e = dinv[src]*dinv[dst]) and aggregation is linear:

    agg1[d] = sum_{e: dst=d} dinv[src_e]*x[src_e] + 2*dinv[d]*x[d]
    u       = relu(dinv[d]*(agg1 @ W1) + b1)
    h2'     = dinv * (u @ W2)            (per-shard table, AllGathered)
    agg2[d] = sum_{e: dst=d} h2'[src_e] + 2*h2'[d]
    out     = log_softmax(dinv[d]*agg2[d] + b2)

Distribution: nodes (rows / dst segments) are sharded over 8 cores.  Layer
1 gathers rows of the raw (fp16) input x, so no cross-core exchange is
needed; layer 2 exchanges the h2' table with one AllGather.

Per-edge aggregation on a core: edges are sorted by dst and padded per
128-dst block; row gathers use the hardware SWDGE dma_gather; segment sums
run on the tensor engine as one-hot matmuls (M matrices streamed from the
host) accumulated per dst-block in PSUM.  dma_scatter_add is NOT used: on
real hardware its read-modify-write pipeline does not accumulate duplicate
rows within one call (last write wins), so all accumulation lives in PSUM.
The self-loop term enters each block as a (2*I) matmul; layer 1 accumulates
feature-major [DIN x dst] directly into the layout the dense W1 matmul
needs, layer 2 accumulates node-major and fuses log_softmax straight out of
PSUM.  No DRAM aggregation tables exist.

Host-side preprocessing is graph-index work: degrees, rsqrt normalizers,
edge sorting/padding, and the one-hot M matrices (values 0/1 in fp16,
identical for both layers).  int16 gather indices limit tables to 32k rows,
so tables are split in two halves (A: src < N/2, B: src >= N/2) with
separate edge streams.
"""

import os
import sys

import numpy as np

sys.path.insert(0, "/opt/trn_rl_repo")

import concourse.bacc as bacc  # noqa: E402
import concourse.bass as bass  # noqa: E402
from concourse import mybir  # noqa: E402
from concourse.bass_utils import run_bass_kernel_spmd  # noqa: E402
from concourse.library_config import mlp as _mlp_lib  # noqa: E402

F32 = mybir.dt.float32
F16 = mybir.dt.float16
I16 = mybir.dt.int16
ALU = mybir.AluOpType
ACT = mybir.ActivationFunctionType
AX = mybir.AxisListType

N = 50000
E = 800000
DIN = 128
H = 128
C = 64
NCORES = 8
BPC = 3  # dst-blocks per gather/M chunk


def _shard_sizes(n):
    shard = n // NCORES
    half = n // 2
    t = (shard + 127) // 128
    return shard, half, t, t * 128


# ----------------------------------------------------------------------------
# Host preprocessing (graph-index work only).
# ----------------------------------------------------------------------------

def _preprocess(edge_index, n):
    """Block-sorted, block-padded edge streams + one-hot M matrices.

    Returns (dinv, per-core input dicts, (NB_A, NB_B)) where NB_h is the
    uniform number of 128-edge batches per dst-block per half."""
    shard, half, T, shard_pad = _shard_sizes(n)
    src = np.asarray(edge_index[0], dtype=np.int64)
    dst = np.asarray(edge_index[1], dtype=np.int64)
    deg = np.bincount(dst, minlength=n).astype(np.float32) + 2.0
    dinv = (1.0 / np.sqrt(deg)).astype(np.float32)

    order = np.argsort(dst, kind="stable")
    ssrc = src[order]
    sdst = dst[order]
    core_bnd = np.searchsorted(sdst, np.arange(NCORES + 1) * shard)

    lists = []
    nb_need = [1, 1]
    for k in range(NCORES):
        lo, hi = core_bnd[k], core_bnd[k + 1]
        cs, cd = ssrc[lo:hi], sdst[lo:hi]
        per_half = []
        for h in (0, 1):
            m = (cs >= half) == (h == 1)
            hs, hd = cs[m], cd[m]
            dl = (hd - k * shard).astype(np.int64)
            o2 = np.argsort(dl, kind="stable")
            hs, dl = hs[o2], dl[o2]
            bnd = np.searchsorted(dl, np.arange(T + 1) * 128)
            cnt = np.diff(bnd)
            if len(cnt):
                nb_need[h] = max(nb_need[h], int((cnt.max() + 127) // 128))
            per_half.append(((hs - h * half).astype(np.int16), dl, dinv[hs], bnd))
        lists.append(per_half)
    NB = (nb_need[0], nb_need[1])

    cores = []
    for k in range(NCORES):
        d = {}
        for h, nm in ((0, "A"), (1, "B")):
            srcrow, dl, dv, bnd = lists[k][h]
            nb = NB[h]
            tot = T * nb * 128
            gflat = np.zeros(tot, np.int16)
            wflat = np.zeros(tot, np.float16)
            dcol = np.zeros(tot, np.int64)
            dsflat = np.zeros(tot, np.float32)
            for b in range(T):
                s, e = int(bnd[b]), int(bnd[b + 1])
                cn = e - s
                base = b * nb * 128
                pos = base + np.arange(cn)
                gflat[pos] = srcrow[s:e]
                wflat[pos] = 1.0
                dcol[pos] = (dl[s:e] - 128 * b) + (pos // 128) * 128
                dsflat[pos] = dv[s:e]
            d["gidx" + nm] = np.tile(
                np.ascontiguousarray(gflat.reshape(-1, 16).T), (8, 1)
            )
            d["dsrc" + nm] = np.ascontiguousarray(dsflat.reshape(-1, 128).T)
            M = np.zeros((128, tot), np.float16)
            kk = np.flatnonzero(wflat)
            M[kk % 128, dcol[kk]] = 1.0
            d["m" + nm] = M
        cores.append(d)
    return dinv, cores, NB


# ----------------------------------------------------------------------------
# Bass kernel.
# ----------------------------------------------------------------------------

def _build(n, NB):
    shard, half, T, shard_pad = _shard_sizes(n)
    NBH = {"A": NB[0], "B": NB[1]}
    GPOS = 1024   # hard HW cap on dma_gather num_idxs
    GSLOTS = 4
    totpos = {h: T * NBH[h] * 128 for h in "AB"}
    NG = {h: (totpos[h] + GPOS - 1) // GPOS for h in "AB"}
    NGMAX = max(NG["A"], NG["B"])

    def npos_call(h, g):
        return min(GPOS, totpos[h] - g * GPOS)

    def bmax(h, g):
        # last dst-block whose positions intersect gather call g
        return min(T - 1, ((g + 1) * GPOS - 1) // (NBH[h] * 128))

    def gneed(h, b):
        # highest gather call needed by block b
        return ((b + 1) * NBH[h] * 128 - 1) // GPOS

    mm_slices = []
    c0 = 0
    while c0 < shard_pad:
        w = min(512, shard_pad - c0)
        mm_slices.append((c0, w))
        c0 += w
    NMM = len(mm_slices)

    nc = bacc.Bacc(None, target_bir_lowering=False, num_devices=NCORES)

    # ---- I/O -------------------------------------------------------------
    xlo = nc.declare_dram_parameter("xlo", [half, DIN], F16, isOutput=False)
    xhi = nc.declare_dram_parameter("xhi", [half, DIN], F16, isOutput=False)
    xown = nc.declare_dram_parameter("xown", [shard_pad, DIN], F16, isOutput=False)
    gidx, dsrc, mbuf = {}, {}, {}
    for h in "AB":
        gidx[h] = nc.declare_dram_parameter(
            f"gidx{h}", [128, T * NBH[h] * 8], I16, isOutput=False
        )
        dsrc[h] = nc.declare_dram_parameter(
            f"dsrc{h}", [128, T * NBH[h]], F32, isOutput=False
        )
        mbuf[h] = nc.declare_dram_parameter(
            f"m{h}", [128, T * NBH[h] * 128], F16, isOutput=False
        )
    dinvown = nc.declare_dram_parameter("dinvown", [128, T], F32, isOutput=False)
    dinvrep = nc.declare_dram_parameter("dinvrep", [128, shard_pad], F16, isOutput=False)
    w1 = nc.declare_dram_parameter("w1", [DIN, H], F32, isOutput=False)
    w2 = nc.declare_dram_parameter("w2", [H, C], F32, isOutput=False)
    b1 = nc.declare_dram_parameter("b1", [H, 1], F32, isOutput=False)
    b2r = nc.declare_dram_parameter("b2r", [128, C], F32, isOutput=False)
    twoI = nc.declare_dram_parameter("twoI", [128, 128], F16, isOutput=False)
    out = nc.declare_dram_parameter("out", [shard, C], F32, isOutput=True)

    # ---- internal DRAM ---------------------------------------------------
    ccin = nc.dram_tensor("ccin", [shard, 128], F16)
    h2full = nc.dram_tensor("h2full", [NCORES * shard, 128], F16, addr_space="Shared")

    # ---- SBUF ------------------------------------------------------------
    A = nc.alloc_sbuf_tensor
    gidx_sb = {h: A(f"gidx{h}_sb", [128, T * NBH[h] * 8], I16) for h in "AB"}
    dsrc_sb = {h: A(f"dsrc{h}_sb", [128, T * NBH[h]], F32) for h in "AB"}
    G = {h: A(f"g{h}", [128, GSLOTS * 1024], F16) for h in "AB"}
    MT = {h: A(f"mt{h}", [128, GSLOTS * GPOS], F16) for h in "AB"}
    xall = A("xall", [128, shard_pad], F16)
    accT = A("accT", [128, shard_pad], F32)
    uT = A("uT", [128, shard_pad], F32)
    h2p = A("h2p", [128, T * 128], F16)
    qmB = A("qmB", [128, T * C], F32)
    dinvrep_sb = A("dinvrep_sb", [128, shard_pad], F16)
    w1_sb = A("w1_sb", [DIN, H], F32)
    w2_sb = A("w2_sb", [H, C], F32)
    b1_sb = A("b1_sb", [H, 1], F32)
    b2r_sb = A("b2r_sb", [128, C], F32)
    twoI_sb = A("twoI_sb", [128, 128], F16)
    dvo_sb = A("dvo_sb", [128, T], F32)
    xt = [A(f"xt{i}", [128, DIN], F16) for i in range(3)]
    qo = [A(f"qo{i}", [128, C], F32) for i in range(3)]
    nmxB = A("nmxB", [128, T], F32)
    smeB = A("smeB", [128, T], F32)
    lnsB = A("lnsB", [128, T], F32)
    qe = A("qe", [128, C], F16)

    pm1 = [nc.alloc_psum_tensor(f"pm1{i}", [128, 128], F32) for i in (0, 1)]
    mmP = [nc.alloc_psum_tensor(f"mmP{i}", [128, 512], F32) for i in (0, 1)]
    h2P = [nc.alloc_psum_tensor(f"h2P{i}", [128, C], F32) for i in (0, 1)]
    pm2 = [nc.alloc_psum_tensor(f"pm2{i}", [128, C], F32) for i in (0, 1)]

    def gcall_view(h, g):
        npos = npos_call(h, g)
        base = (g % GSLOTS) * 1024
        return G[h][:, base : base + (npos // 128) * 128].rearrange(
            "p (s e) -> p s e", e=128
        )

    def g_batch(h, q):
        g = q // 8
        base = (g % GSLOTS) * 1024 + (q % 8) * 128
        return G[h][:, base : base + 128]

    def m_batch(h, q):
        g = q // 8
        base = (g % GSLOTS) * GPOS + (q % 8) * 128
        return MT[h][:, base : base + 128]

    # ---- static schedules (1 sem inc per instruction on ve/pe/ac) -------
    ve_xt = [t + 1 for t in range(T)]
    _b = T
    ve_scale = {}
    for g in range(NGMAX):
        for h in "AB":
            if g < NG[h]:
                _b += 1
                ve_scale[(g, h)] = _b
    ve_acc = [_b + b + 1 for b in range(T)]
    _b += T
    ve_h2 = [_b + t + 1 for t in range(T)]
    _b += T
    ve_qm = [_b + 2 * b + 1 for b in range(T)]
    ve_negmax = [_b + 2 * b + 2 for b in range(T)]
    _b += 2 * T
    ve_out = [_b + b + 1 for b in range(T)]
    VE_END = _b + T

    BL1 = 1 + NBH["A"] + NBH["B"]
    pe_blk1 = [(b + 1) * BL1 for b in range(T)]
    _p = T * BL1
    pe_mm = [_p + j + 1 for j in range(NMM)]
    _p += NMM
    pe_h2 = [_p + t + 1 for t in range(T)]
    _p += T
    pe_blk2 = [_p + (b + 1) * BL1 for b in range(T)]
    PE_END = _p + T * BL1

    ac_copy = [b + 1 for b in range(T)]
    ac_relu = [T + j + 1 for j in range(NMM)]
    ac_ln = [T + NMM + 2 * (b + 1) for b in range(T)]
    AC_END = T + NMM + 2 * T

    NPRE = 11
    LD_PRE = 16 * NPRE
    W_CCIN = 16 * T

    GV, MV = {}, {}
    gcnt = {(h, sl): 0 for h in "AB" for sl in range(GSLOTS)}
    mcnt = {(h, sl): 0 for h in "AB" for sl in range(GSLOTS)}
    counters = {}

    def mk_counter(name):
        counters[name] = 0

        def bump(inst, sem_h, d):
            counters[name] += d
            inst.then_inc(sem_h, d)
            return counters[name]

        return bump

    def rows(t):
        r0 = t * 128
        return r0, min(r0 + 128, shard)

    from contextlib import ExitStack

    with ExitStack() as _st:
        block = _st.enter_context(nc.Block())
        sem = lambda nm: _st.enter_context(nc.semaphore(nm))
        ld_pre = sem("ld_pre")
        w_ccin = sem("w_ccin")
        xq = [sem(f"xq{i}") for i in range(3)]
        w_out = [sem(f"w_out{i}") for i in range(3)]
        gq = {h: [sem(f"g{h}{i}") for i in range(GSLOTS)] for h in "AB"}
        mq = {h: [sem(f"m{h}{i}") for i in range(GSLOTS)] for h in "AB"}
        ve = sem("ve")
        pe = sem("pe")
        ac = sem("ac")
        cc = sem("cc")

        # --------------------------------------------------------- gpsimd
        @block.gpsimd
        def _(gp: bass.BassGpSimd):
            gp.load_library(_mlp_lib)
            gp.wait_ge(ld_pre, LD_PRE)
            for li in range(2):
                if li == 1:
                    gp.wait_ge(w_ccin, W_CCIN)
                    gp.collective_compute(
                        "AllGather",
                        ALU.bypass,
                        replica_groups=[list(range(NCORES))],
                        ins=[ccin[:]],
                        outs=[h2full[:]],
                    ).then_inc(cc, 1)
                    gp.wait_ge(cc, 1)
                tabs = {
                    "A": xlo if li == 0 else h2full[: NCORES * shard // 2, :],
                    "B": xhi if li == 0 else h2full[NCORES * shard // 2 :, :],
                }
                for g in range(NGMAX):
                    for h in "AB":
                        if g >= NG[h]:
                            continue
                        if li == 0 and g >= GSLOTS:
                            gp.wait_ge(pe, pe_blk1[bmax(h, g - GSLOTS)])
                        elif li == 1 and g >= GSLOTS:
                            gp.wait_ge(pe, pe_blk2[bmax(h, g - GSLOTS)])
                        npos = npos_call(h, g)
                        gcnt[(h, g % GSLOTS)] += 16
                        GV[(li, g, h)] = gcnt[(h, g % GSLOTS)]
                        gp.dma_gather(
                            out_ap=gcall_view(h, g),
                            in_ap=tabs[h][:],
                            idxs_ap=gidx_sb[h][
                                :, g * GPOS // 16 : g * GPOS // 16 + npos // 16
                            ],
                            num_idxs=npos,
                            num_idxs_reg=npos,
                            elem_size=128,
                        ).then_inc(gq[h][g % GSLOTS], 16)

        # ----------------------------------------------------------- sync
        @block.sync
        def _(sp: bass.BassEngine):
            preloads = [
                (gidx_sb["A"][:], gidx["A"][:]), (gidx_sb["B"][:], gidx["B"][:]),
                (dsrc_sb["A"][:], dsrc["A"][:]), (dsrc_sb["B"][:], dsrc["B"][:]),
                (w1_sb[:], w1[:]), (w2_sb[:], w2[:]), (b1_sb[:], b1[:]),
                (b2r_sb[:], b2r[:]), (twoI_sb[:], twoI[:]),
                (dvo_sb[:], dinvown[:]), (dinvrep_sb[:], dinvrep[:]),
            ]
            assert len(preloads) == NPRE
            for o_, i_ in preloads:
                sp.dma_start(out=o_, in_=i_).then_inc(ld_pre, 16)
            for t in range(T):
                if t >= 3:
                    sp.wait_ge(ve, ve_xt[t - 3])  # WAR xt slot
                sp.dma_start(
                    out=xt[t % 3][:], in_=xown[t * 128 : (t + 1) * 128, :]
                ).then_inc(xq[t % 3], 16)

            def m_loads(li):
                for g in range(NGMAX):
                    for h in "AB":
                        if g >= NG[h]:
                            continue
                        if li == 1 and g < GSLOTS:
                            sp.wait_ge(pe, pe_blk1[T - 1])
                        elif g >= GSLOTS:
                            pv = (pe_blk1 if li == 0 else pe_blk2)[
                                bmax(h, g - GSLOTS)
                            ]
                            sp.wait_ge(pe, pv)
                        npos = npos_call(h, g)
                        base = (g % GSLOTS) * GPOS
                        mcnt[(h, g % GSLOTS)] += 16
                        MV[(li, g, h)] = mcnt[(h, g % GSLOTS)]
                        sp.dma_start(
                            out=MT[h][:, base : base + npos],
                            in_=mbuf[h][:, g * GPOS : g * GPOS + npos],
                        ).then_inc(mq[h][g % GSLOTS], 16)

            m_loads(0)
            # ccin writes MUST precede the layer-2 M loads: the l2 g>=GSLOTS
            # load waits on L2 PE progress, which needs the collective, which
            # needs these writes (SP is in-order).
            for t in range(T):
                r0, r1 = rows(t)
                sp.wait_ge(ve, ve_h2[t])
                sp.dma_start(
                    out=ccin[r0:r1, :], in_=h2p[: r1 - r0, t * 128 : (t + 1) * 128]
                ).then_inc(w_ccin, 16)
            m_loads(1)
            for b in range(T):
                r0, r1 = rows(b)
                sp.wait_ge(ve, ve_out[b])
                sp.dma_start(out=out[r0:r1, :], in_=qo[b % 3][: r1 - r0, :]).then_inc(
                    w_out[b % 3], 16
                )
            for sl in range(3):
                cnt = len([b for b in range(T) if b % 3 == sl])
                if cnt:
                    sp.wait_ge(w_out[sl], 16 * cnt)

        # --------------------------------------------------------- vector
        @block.vector
        def _(vec: bass.BassVectorEngine):
            bump = mk_counter("ve")

            def vinc(inst):
                return bump(inst, ve, 1)

            vec.wait_ge(ld_pre, LD_PRE)
            vec.memset(h2p[:], 0.0)  # uncounted; h2 tiles only fill cols 0..C
            for t in range(T):
                vec.wait_ge(xq[t % 3], 16 * (t // 3 + 1))
                vinc(
                    vec.tensor_tensor(
                        out=xall[:, t * 128 : (t + 1) * 128],
                        in0=xt[t % 3][:],
                        in1=dvo_sb[:, t : t + 1].to_broadcast([128, DIN]),
                        op=ALU.mult,
                    )
                )
                assert counters["ve"] == ve_xt[t]
            for g in range(NGMAX):
                for h in "AB":
                    if g >= NG[h]:
                        continue
                    vec.wait_ge(gq[h][g % GSLOTS], GV[(0, g, h)])
                    npos = npos_call(h, g)
                    gv = gcall_view(h, g)
                    vinc(
                        vec.tensor_tensor(
                            out=gv,
                            in0=gv,
                            in1=dsrc_sb[h][
                                :, g * 8 : g * 8 + npos // 128
                            ].to_broadcast([128, npos // 128, 128]),
                            op=ALU.mult,
                        )
                    )
                    assert counters["ve"] == ve_scale[(g, h)]
            for b in range(T):
                vec.wait_ge(ac, ac_copy[b])
                sl = slice(b * 128, (b + 1) * 128)
                vinc(
                    vec.tensor_tensor(
                        out=accT[:, sl], in0=accT[:, sl], in1=dinvrep_sb[:, sl],
                        op=ALU.mult,
                    )
                )
                assert counters["ve"] == ve_acc[b]
            for t in range(T):
                vec.wait_ge(pe, pe_h2[t])
                vinc(
                    vec.tensor_tensor(
                        out=h2p[:, t * 128 : t * 128 + C],
                        in0=h2P[t % 2][:],
                        in1=dvo_sb[:, t : t + 1].to_broadcast([128, C]),
                        op=ALU.mult,
                    )
                )
                assert counters["ve"] == ve_h2[t]
            for b in range(T):
                vec.wait_ge(pe, pe_blk2[b])
                qm = qmB[:, b * C : (b + 1) * C]
                vinc(
                    vec.scalar_tensor_tensor(
                        out=qm, in0=pm2[b % 2][:], scalar=dvo_sb[:, b : b + 1],
                        in1=b2r_sb[:], op0=ALU.mult, op1=ALU.add,
                    )
                )
                assert counters["ve"] == ve_qm[b]
                vec.drain()
                vinc(
                    vec.tensor_reduce(
                        out=nmxB[:, b : b + 1], in_=qm, axis=AX.X, op=ALU.max,
                        negate=True,
                    )
                )
                assert counters["ve"] == ve_negmax[b]
            for b in range(T):
                vec.wait_ge(ac, ac_ln[b])
                if b >= 3:
                    vec.wait_ge(w_out[b % 3], 16 * (b // 3))  # WAR qo slot
                vinc(
                    vec.scalar_tensor_tensor(
                        out=qo[b % 3][:],
                        in0=qmB[:, b * C : (b + 1) * C],
                        scalar=lnsB[:, b : b + 1],
                        in1=nmxB[:, b : b + 1].to_broadcast([128, C]),
                        op0=ALU.subtract, op1=ALU.add,
                    )
                )
                assert counters["ve"] == ve_out[b]
            assert counters["ve"] == VE_END

        # --------------------------------------------------------- tensor
        @block.tensor
        def _(te: bass.BassTensorEngine):
            bump = mk_counter("pe")

            def pinc(inst):
                return bump(inst, pe, 1)

            te.wait_ge(ld_pre, LD_PRE)
            gwaited = {h: -1 for h in "AB"}
            for b in range(T):
                if b >= 2:
                    te.wait_ge(ac, ac_copy[b - 2])  # WAR pm1 slot
                te.wait_ge(ve, ve_xt[b])
                for h in "AB":
                    while gwaited[h] < gneed(h, b):
                        gwaited[h] += 1
                        g = gwaited[h]
                        te.wait_ge(ve, ve_scale[(g, h)])
                        te.wait_ge(mq[h][g % GSLOTS], MV[(0, g, h)])
                pinc(
                    te.matmul(
                        out=pm1[b % 2][:],
                        lhsT=xall[:, b * 128 : (b + 1) * 128],
                        rhs=twoI_sb[:],
                        start=True,
                        stop=False,
                    )
                )
                for h in "AB":
                    nb = NBH[h]
                    for j in range(nb):
                        q = b * nb + j
                        pinc(
                            te.matmul(
                                out=pm1[b % 2][:],
                                lhsT=g_batch(h, q),
                                rhs=m_batch(h, q),
                                start=False,
                                stop=(h == "B" and j == nb - 1),
                            )
                        )
                assert counters["pe"] == pe_blk1[b]
            for j, (c0, w) in enumerate(mm_slices):
                te.wait_ge(ve, ve_acc[(c0 + w - 1) // 128])
                if j >= 2:
                    te.wait_ge(ac, ac_relu[j - 2])  # WAR mmP slot
                pinc(
                    te.matmul(
                        out=mmP[j % 2][:, :w],
                        lhsT=w1_sb[:],
                        rhs=accT[:, c0 : c0 + w],
                        start=True,
                        stop=True,
                    )
                )
                assert counters["pe"] == pe_mm[j]
            for t in range(T):
                j_need = ((t + 1) * 128 - 1) // 512
                te.wait_ge(ac, ac_relu[min(j_need, NMM - 1)])
                if t >= 2:
                    te.wait_ge(ve, ve_h2[t - 2])  # WAR h2P slot
                pinc(
                    te.matmul(
                        out=h2P[t % 2][:],
                        lhsT=uT[:, t * 128 : (t + 1) * 128],
                        rhs=w2_sb[:],
                        start=True,
                        stop=True,
                    )
                )
                assert counters["pe"] == pe_h2[t]
            gwaited = {h: -1 for h in "AB"}
            for b in range(T):
                if b >= 2:
                    te.wait_ge(ve, ve_qm[b - 2])  # WAR pm2 slot
                te.wait_ge(ve, ve_h2[b])
                for h in "AB":
                    while gwaited[h] < gneed(h, b):
                        gwaited[h] += 1
                        g = gwaited[h]
                        te.wait_ge(gq[h][g % GSLOTS], GV[(1, g, h)])
                        te.wait_ge(mq[h][g % GSLOTS], MV[(1, g, h)])
                pinc(
                    te.matmul(
                        out=pm2[b % 2][:],
                        lhsT=twoI_sb[:],
                        rhs=h2p[:, b * 128 : b * 128 + C],
                        start=True,
                        stop=False,
                    )
                )
                for h in "AB":
                    nb = NBH[h]
                    for j in range(nb):
                        q = b * nb + j
                        pinc(
                            te.matmul(
                                out=pm2[b % 2][:],
                                lhsT=m_batch(h, q),
                                rhs=g_batch(h, q)[:, :C],
                                start=False,
                                stop=(h == "B" and j == nb - 1),
                            )
                        )
                assert counters["pe"] == pe_blk2[b]
            assert counters["pe"] == PE_END

        # --------------------------------------------------------- scalar
        @block.scalar
        def _(sc: bass.BassScalarEngine):
            bump = mk_counter("ac")
            sc.wait_ge(ld_pre, LD_PRE)
            for b in range(T):
                sc.wait_ge(pe, pe_blk1[b])
                bump(
                    sc.activation(
                        out=accT[:, b * 128 : (b + 1) * 128],
                        in_=pm1[b % 2][:],
                        func=ACT.Copy,
                    ),
                    ac, 1,
                )
                assert counters["ac"] == ac_copy[b]
            for j, (c0, w) in enumerate(mm_slices):
                sc.wait_ge(pe, pe_mm[j])
                bump(
                    sc.activation(
                        out=uT[:, c0 : c0 + w],
                        in_=mmP[j % 2][:, :w],
                        func=ACT.Relu,
                        bias=b1_sb[:],
                    ),
                    ac, 1,
                )
                assert counters["ac"] == ac_relu[j]
            for b in range(T):
                sc.wait_ge(ve, ve_negmax[b])
                bump(
                    sc.activation(
                        out=qe[:],
                        in_=qmB[:, b * C : (b + 1) * C],
                        func=ACT.Exp,
                        bias=nmxB[:, b : b + 1],
                        accum_out=smeB[:, b : b + 1],
                    ),
                    ac, 1,
                )
                sc.drain()
                bump(
                    sc.activation(
                        out=lnsB[:, b : b + 1], in_=smeB[:, b : b + 1], func=ACT.Ln
                    ),
                    ac, 1,
                )
                assert counters["ac"] == ac_ln[b]
            assert counters["ac"] == AC_END

    nc.compile()
    return nc


# ----------------------------------------------------------------------------
# Public entry point.
# ----------------------------------------------------------------------------

_CACHE = {}
LAST_RESULTS = None  # BassKernelResults from the most recent traced run


def _get_kernel(n, NB):
    key = (n, NB)
    if key not in _CACHE:
        _CACHE[key] = _build(n, NB)
    return _CACHE[key]


def _in_maps(x, W1, b1, W2, b2, dinv, cores, n):
    shard, half, T, shard_pad = _shard_sizes(n)
    x16 = x.astype(np.float16)
    xlo = np.ascontiguousarray(x16[:half])
    xhi = np.ascontiguousarray(x16[half:])
    b2r = np.tile(np.asarray(b2, np.float32)[None, :], (128, 1))
    twoI = (2.0 * np.eye(128)).astype(np.float16)
    maps = []
    for k in range(NCORES):
        xo = np.zeros((shard_pad, DIN), np.float16)
        xo[:shard] = x16[k * shard : (k + 1) * shard]
        dvp = np.zeros(shard_pad, np.float32)
        dvp[:shard] = dinv[k * shard : (k + 1) * shard]
        dvo = np.ascontiguousarray(dvp.reshape(T, 128).T)
        drep = np.tile(dvp.astype(np.float16)[None, :], (128, 1))
        m = dict(
            xlo=xlo, xhi=xhi, xown=xo, dinvown=dvo, dinvrep=drep,
            w1=np.asarray(W1, np.float32), w2=np.asarray(W2, np.float32),
            b1=np.asarray(b1, np.float32).reshape(H, 1), b2r=b2r, twoI=twoI,
        )
        m.update(cores[k])
        maps.append(m)
    return maps


def kernel(x, edge_index, W1, b1, W2, b2):
    n = x.shape[0]
    x = np.ascontiguousarray(np.asarray(x, dtype=np.float32))
    dinv, cores, NB = _preprocess(edge_index, n)
    nc = _get_kernel(n, NB)
    maps = _in_maps(x, W1, b1, W2, b2, dinv, cores, n)

    if os.environ.get("KERNEL_SIM"):
        from concourse import bass_interp

        sim = bass_interp.MultiCoreSim(nc, NCORES)
        for k in range(NCORES):
            for kk, vv in maps[k].items():
                sim.cores[k].tensor(kk)[:] = vv
        sim.simulate()
        outs = [np.array(sim.cores[k].tensor("out")) for k in range(NCORES)]
    else:
        global LAST_RESULTS
        trace = bool(os.environ.get("KERNEL_TRACE"))
        kwargs = {}
        if trace:
            tmpdir = os.environ.get("KERNEL_TRACE_DIR") or None
            if tmpdir:
                os.makedirs(tmpdir, exist_ok=True)
            kwargs = dict(trace=True, tmpdir=tmpdir)
        res = run_bass_kernel_spmd(nc, maps, list(range(NCORES)), **kwargs)
        if trace:
            LAST_RESULTS = res
        outs = [res.results[k]["out"] for k in range(NCORES)]
    return np.concatenate(outs, axis=0)

